# revision 15
# baseline (speedup 1.0000x reference)
"""Trainium2 Bass kernel for nn_Attend (softclamped masked attention, returns out+attn).

Problem (hardcoded): q,k,v [2,16,2048,64] f32, mask [2,2048] bool.
  sim  = (q @ k^T) / sqrt(64)
  sim  = tanh(sim / 50) * 50              (Gemma2-style softcap)
  sim  = where(mask_j, sim, -FLT_MAX)
  attn = softmax(sim, axis=-1)            -> output [2,16,2048,2048]
  out  = attn @ v                         -> output [2,16,2048,64]

Sharding: batch*heads (32) split across 8 cores, 4 heads/core, no cross-core comms.
Host prep: q/k are passed per-head transposed ([d, i] / [d, j]) and the mask as a
per-partition additive bias table; both are cheap input-layout choices.

Per-core device pipeline (per head):
  - qT_r/kT_r: DVE-rounded to f32r (12-bit mantissa, full-rate PE streams).
    kT lives twice (partition halves 0-63 / 64-127) so two j-tiles run
    CONCURRENTLY in the PE array via tile_position row-tiling (K=64 each).
  - simT[j,i] = kT_r.T @ qT_r   (f32r, N=512, pairs of i-chunks share a PSUM
    [128,1024] tile so the ACT ops run 1024-wide)
  - t = tanh(simT/400 + mask_bias[j])  (mask folded into per-partition ACT bias:
    masked j -> tanh = -1 -> e = exp(-50) ~ 2e-22 ~ 0)
  - e = exp(50 t) stored BF16 (one ACT pass; feeds matmul2, row-sums and the
    attn transposes; the f32r logit path keeps exponent-sensitive error small)
  - outT[(d|1), i] += [v|1s]_bf16.T @ e_bf16  (ones column -> softmax row-sums free)
  - attn[i,j] = PE-transpose(e_bf16) * (1/rowsum)  (DVE tensor_scalar from PSUM)
  - out[i,d]  = PE-transpose(outT) * (1/rowsum)
"""
import numpy as np

import concourse.mybir as mybir
import concourse.tile as tile
from concourse import bacc
from concourse.bass_utils import run_bass_kernel_spmd
from concourse.masks import make_identity

B, H, S, D = 2, 16, 2048, 64
NCORES = 8
HPC = (B * H) // NCORES      # 4 heads per core
PT = 128                     # partition tile
JT = S // PT                 # 16 j-tiles per head
ICS = 512                    # i-chunk size (matmul N)
NIC = S // ICS               # 4 i-chunks
IBC = ICS // PT              # 4 i-blocks per chunk
MASK_BIAS = -20.0

F32 = mybir.dt.float32
F32R = mybir.dt.float32r
BF16 = mybir.dt.bfloat16
Tanh = mybir.ActivationFunctionType.Tanh
Exp = mybir.ActivationFunctionType.Exp

_NC_CACHE = None


def _build():
    nc = bacc.Bacc("TRN2", target_bir_lowering=False, debug=False)
    qT_d = nc.dram_tensor("qT", [HPC, D, S], F32, kind="ExternalInput")
    kT_d = nc.dram_tensor("kT", [HPC, D, S], F32, kind="ExternalInput")
    v_d = nc.dram_tensor("v", [HPC, S, D], F32, kind="ExternalInput")
    mb_d = nc.dram_tensor("mb", [HPC, PT, JT], F32, kind="ExternalInput")
    attn_d = nc.dram_tensor("attn", [HPC, S, S], F32, kind="ExternalOutput")
    out_d = nc.dram_tensor("out", [HPC, S, D], F32, kind="ExternalOutput")

    with tile.TileContext(nc) as tc:
        import contextlib
        ctx = contextlib.ExitStack()
        with ctx:
            singles = ctx.enter_context(tc.tile_pool(name="singles", bufs=1))
            stage = ctx.enter_context(tc.tile_pool(name="stage", bufs=2))
            proj = ctx.enter_context(tc.tile_pool(name="proj", bufs=2))
            er_pool = ctx.enter_context(tc.tile_pool(name="er", bufs=2))
            work = ctx.enter_context(tc.tile_pool(name="work", bufs=2))
            outs = ctx.enter_context(tc.tile_pool(name="outs", bufs=2))
            recips = ctx.enter_context(tc.tile_pool(name="recips", bufs=2 * IBC))
            attn_out = ctx.enter_context(tc.tile_pool(name="attn_out", bufs=2))
            ps_mm = ctx.enter_context(tc.tile_pool(name="ps_mm", bufs=2, space="PSUM"))
            ps_acc = ctx.enter_context(tc.tile_pool(name="ps_acc", bufs=2, space="PSUM"))
            ps_attn = ctx.enter_context(tc.tile_pool(name="ps_attn", bufs=2, space="PSUM"))
            ps_tr = ps_attn

            ident_bf = singles.tile([PT, PT], BF16)
            make_identity(nc, ident_bf)
            ident65 = singles.tile([D + 1, D + 1], F32)
            make_identity(nc, ident65)

            for h in range(HPC):
                # ---- load + round projections ----
                qT_f = stage.tile([D, S], F32, tag="qkT_f")
                nc.sync.dma_start(out=qT_f, in_=qT_d[h])
                qT_r = proj.tile([D, S], F32R, tag="qT_r")
                nc.gpsimd.tensor_copy(out=qT_r, in_=qT_f)

                kT_f = stage.tile([D, S], F32, tag="qkT_f")
                nc.sync.dma_start(out=kT_f, in_=kT_d[h])
                kT_r = proj.tile([D, S], F32R, tag="kT_r")
                nc.gpsimd.tensor_copy(out=kT_r, in_=kT_f)

                # v as bf16 with ones column
                v_f = stage.tile([PT, JT, D], F32, tag="v_f")
                nc.sync.dma_start(
                    out=v_f, in_=v_d[h].rearrange("(t p) d -> p t d", p=PT)
                )
                v_aug = proj.tile([PT, JT, D + 1], BF16, tag="v_aug")
                nc.gpsimd.tensor_copy(out=v_aug[:, :, 0:D], in_=v_f)
                nc.vector.memset(v_aug[:, :, D:D + 1], 1.0)

                mb_sb = stage.tile([PT, JT], F32, tag="mb_sb")
                nc.sync.dma_start(out=mb_sb, in_=mb_d[h])

                for icp in range(2):          # pairs of i-chunks
                    e_r = er_pool.tile([PT, JT, 2 * ICS], BF16, tag="e_r")
                    for g in range(JT // 2):
                        tT2 = work.tile([PT, 4 * ICS], F32, tag="tT2")
                        for half in range(2):
                            jt = 2 * g + half
                            pt = ps_mm.tile([PT, 2 * ICS], F32, tag="simT")
                            for ici in range(2):
                                ic = 2 * icp + ici
                                nc.tensor.matmul(
                                    pt[:, ici * ICS:(ici + 1) * ICS],
                                    kT_r[:, jt * PT:(jt + 1) * PT],
                                    qT_r[:, ic * ICS:(ic + 1) * ICS],
                                    start=True, stop=True,
                                )
                            nc.scalar.activation(
                                tT2[:, half * 2 * ICS:(half + 1) * 2 * ICS], pt, Tanh,
                                bias=mb_sb[:, jt:jt + 1], scale=1.0 / 400.0,
                            )
                        nc.scalar.activation(
                            e_r[:, 2 * g:2 * g + 2, :].rearrange("p a b -> p (a b)"),
                            tT2, Exp, scale=50.0,
                        )

                    outT_pair = []
                    for _i in range(2):
                        outT_p = ps_acc.tile([D + 1, ICS], F32, tag="outT")
                        outT_pair.append(outT_p)
                    for jt in range(JT):
                        for ici in range(2):
                            esl = slice(ici * ICS, (ici + 1) * ICS)
                            nc.tensor.matmul(
                                outT_pair[ici], v_aug[:, jt, :], e_r[:, jt, esl],
                                start=(jt == 0), stop=(jt == JT - 1),
                            )
                    for ici in range(2):
                        ic = 2 * icp + ici
                        isl = slice(ic * ICS, (ic + 1) * ICS)
                        esl = slice(ici * ICS, (ici + 1) * ICS)
                        outT_ps = outT_pair[ici]
                        outT_sb = outs.tile([D + 1, ICS], F32, tag="outT_sb")
                        nc.vector.tensor_copy(outT_sb, outT_ps)

                        out_sb = outs.tile([PT, IBC, D], F32, tag="out_sb")
                        rc_list = []
                        for ib in range(IBC):
                            outtr = ps_tr.tile([PT, D + 1], F32, tag="attn_ps")
                            nc.tensor.transpose(
                                outtr,
                                outT_sb[:, ib * PT:(ib + 1) * PT],
                                ident65,
                            )
                            rc = recips.tile([PT, 1], F32, tag="recip")
                            nc.vector.reciprocal(rc, outtr[:, D:D + 1])
                            nc.vector.tensor_scalar_mul(
                                out=out_sb[:, ib, :], in0=outtr[:, 0:D], scalar1=rc
                            )
                            rc_list.append(rc)
                        nc.sync.dma_start(
                            out=out_d[h, isl, :].rearrange("(b p) d -> p b d", p=PT),
                            in_=out_sb,
                        )

                        # ---- attn writeout: bf16 transposes + normalize ----
                        for ibp in range(IBC // 2):
                            attn_sb = attn_out.tile([PT, 2, S], F32, tag="attn_sb")
                            for sub in range(2):
                                ib = 2 * ibp + sub
                                for half2 in range(2):
                                    attn_ps = ps_attn.tile([PT, 8 * PT], BF16, tag="attn_ps")
                                    for jj in range(8):
                                        jt = half2 * 8 + jj
                                        nc.tensor.transpose(
                                            attn_ps[:, jj * PT:(jj + 1) * PT],
                                            e_r[:, jt, ici * ICS + ib * PT:
                                                ici * ICS + (ib + 1) * PT],
                                            ident_bf,
                                        )
                                    nc.vector.tensor_scalar_mul(
                                        out=attn_sb[:, sub,
                                                    half2 * 8 * PT:(half2 + 1) * 8 * PT],
                                        in0=attn_ps,
                                        scalar1=rc_list[ib],
                                    )
                            row0 = ic * ICS + 2 * ibp * PT
                            nc.sync.dma_start(
                                out=attn_d[h, row0:row0 + 2 * PT, :].rearrange(
                                    "(b p) j -> p b j", p=PT),
                                in_=attn_sb,
                            )
    nc.compile()
    return nc


def _get_nc():
    global _NC_CACHE
    if _NC_CACHE is None:
        _NC_CACHE = _build()
    return _NC_CACHE


LAST_EXEC_NS = None


def kernel(q, k, v, mask, _trace=False):
    global LAST_EXEC_NS
    q = np.ascontiguousarray(np.asarray(q, dtype=np.float32)).reshape(B * H, S, D)
    k = np.ascontiguousarray(np.asarray(k, dtype=np.float32)).reshape(B * H, S, D)
    v = np.ascontiguousarray(np.asarray(v, dtype=np.float32)).reshape(B * H, S, D)
    mask = np.asarray(mask).astype(bool)

    qT = np.ascontiguousarray(q.transpose(0, 2, 1))                  # [BH, D, S]
    kT = np.ascontiguousarray(k.transpose(0, 2, 1))                  # [BH, D, S]

    # mask bias per head, laid out [128, JT] partition-major per j-tile
    mb = np.where(mask, np.float32(0.0), np.float32(MASK_BIAS))      # [B, S]
    mb = mb.reshape(B, JT, PT).transpose(0, 2, 1)                    # [B, 128, JT]
    mb = np.ascontiguousarray(mb, dtype=np.float32)

    in_maps = []
    for c in range(NCORES):
        heads = range(c * HPC, (c + 1) * HPC)
        in_maps.append({
            "qT": qT[c * HPC:(c + 1) * HPC],
            "kT": kT[c * HPC:(c + 1) * HPC],
            "v": v[c * HPC:(c + 1) * HPC],
            "mb": np.stack([mb[hh // H] for hh in heads]),
        })

    nc = _get_nc()
    res = run_bass_kernel_spmd(
        nc, in_maps, core_ids=list(range(NCORES)), trace=_trace
    )
    LAST_EXEC_NS = res.exec_time_ns

    attn = np.concatenate([r["attn"] for r in res.results]).reshape(B, H, S, S)
    out = np.concatenate([r["out"] for r in res.results]).reshape(B, H, S, D)
    return out, attn


# revision 16
# speedup vs baseline: 1.2063x; 1.2063x over previous
"""Trainium2 Bass kernel for nn_Attend (softclamped masked attention, returns out+attn).

Problem (hardcoded): q,k,v [2,16,2048,64] f32, mask [2,2048] bool.
  sim  = (q @ k^T) / sqrt(64)
  sim  = tanh(sim / 50) * 50              (Gemma2-style softcap)
  sim  = where(mask_j, sim, -FLT_MAX)
  attn = softmax(sim, axis=-1)            -> output [2,16,2048,2048]
  out  = attn @ v                         -> output [2,16,2048,64]

Sharding: batch*heads (32) split across 8 cores, 4 heads/core, no cross-core comms.
Host prep: q/k are passed per-head transposed ([d, i] / [d, j]) and the mask as a
per-partition additive bias table; both are cheap input-layout choices.

Per-core device pipeline (per head):
  - qT_r/kT_r: rounded to f32r on GpSimd (12-bit mantissa; exact-exponent path).
  - simT[j,i] = kT_r.T @ qT_r   (f32r, N=512; pairs of i-chunks share a PSUM
    [128,1024] tile so the ACT ops run 1024-wide and kT weights load once per pair)
  - t = tanh(simT/400 + mask_bias[j])  (mask folded into per-partition ACT bias:
    masked j -> tanh = -1 -> e = exp(-50) ~ 2e-22 ~ 0)
  - e = exp(50 t) stored BF16 (one ACT pass; feeds matmul2, row-sums and the
    attn transposes; the f32r logit path keeps exponent-sensitive error small)
  - outT[(d|1), i] += [v|1s]_bf16.T @ e_bf16  (ones column -> softmax row-sums free)
  - attn[i,j] = PE-transpose(e_bf16) * (1/rowsum)  (DVE tensor_scalar from PSUM)
  - out[i,d]  = PE-transpose(outT) * (1/rowsum)
"""
import numpy as np

import concourse.mybir as mybir
import concourse.tile as tile
from concourse import bacc
from concourse.bass_utils import run_bass_kernel_spmd
from concourse.masks import make_identity

B, H, S, D = 2, 16, 2048, 64
NCORES = 8
HPC = (B * H) // NCORES      # 4 heads per core
PT = 128                     # partition tile
JT = S // PT                 # 16 j-tiles per head
ICS = 512                    # i-chunk size (matmul N)
NIC = S // ICS               # 4 i-chunks
IBC = ICS // PT              # 4 i-blocks per chunk
MASK_BIAS = -20.0

F32 = mybir.dt.float32
F32R = mybir.dt.float32r
BF16 = mybir.dt.bfloat16
Tanh = mybir.ActivationFunctionType.Tanh
Exp = mybir.ActivationFunctionType.Exp

_NC_CACHE = None


def _build():
    nc = bacc.Bacc("TRN2", target_bir_lowering=False, debug=False)
    qT_d = nc.dram_tensor("qT", [HPC, D, S], F32, kind="ExternalInput")
    kT_d = nc.dram_tensor("kT", [HPC, D, S], F32, kind="ExternalInput")
    v_d = nc.dram_tensor("v", [HPC, S, D], F32, kind="ExternalInput")
    mb_d = nc.dram_tensor("mb", [HPC, PT, JT], F32, kind="ExternalInput")
    attn_d = nc.dram_tensor("attn", [HPC, S, S], F32, kind="ExternalOutput")
    out_d = nc.dram_tensor("out", [HPC, S, D], F32, kind="ExternalOutput")

    with tile.TileContext(nc) as tc:
        import contextlib
        ctx = contextlib.ExitStack()
        with ctx:
            singles = ctx.enter_context(tc.tile_pool(name="singles", bufs=1))
            stage = ctx.enter_context(tc.tile_pool(name="stage", bufs=2))
            proj = ctx.enter_context(tc.tile_pool(name="proj", bufs=2))
            er_pool = ctx.enter_context(tc.tile_pool(name="er", bufs=2))
            work = ctx.enter_context(tc.tile_pool(name="work", bufs=3))
            outs = ctx.enter_context(tc.tile_pool(name="outs", bufs=2))
            recips = ctx.enter_context(tc.tile_pool(name="recips", bufs=2 * IBC))
            attn_out = ctx.enter_context(tc.tile_pool(name="attn_out", bufs=2))
            ps_mm = ctx.enter_context(tc.tile_pool(name="ps_mm", bufs=2, space="PSUM"))
            ps_acc = ctx.enter_context(tc.tile_pool(name="ps_acc", bufs=2, space="PSUM"))
            ps_attn = ctx.enter_context(tc.tile_pool(name="ps_attn", bufs=2, space="PSUM"))
            ps_tr = ps_attn

            ident_bf = singles.tile([PT, PT], BF16)
            make_identity(nc, ident_bf)
            ident65 = singles.tile([D + 1, D + 1], F32)
            make_identity(nc, ident65)

            for h in range(HPC):
                # ---- load + round projections ----
                qT_f = stage.tile([D, S], F32, tag="qkT_f")
                nc.sync.dma_start(out=qT_f, in_=qT_d[h])
                qT_r = proj.tile([D, S], F32R, tag="qT_r")
                nc.gpsimd.tensor_copy(out=qT_r, in_=qT_f)

                kT_f = stage.tile([D, S], F32, tag="qkT_f")
                nc.sync.dma_start(out=kT_f, in_=kT_d[h])
                kT_r = proj.tile([D, S], F32R, tag="kT_r")
                nc.gpsimd.tensor_copy(out=kT_r, in_=kT_f)

                # v as bf16 with ones column
                v_f = stage.tile([PT, JT, D], F32, tag="v_f")
                nc.sync.dma_start(
                    out=v_f, in_=v_d[h].rearrange("(t p) d -> p t d", p=PT)
                )
                v_aug = proj.tile([PT, JT, D + 1], BF16, tag="v_aug")
                nc.gpsimd.tensor_copy(out=v_aug[:, :, 0:D], in_=v_f)
                nc.vector.memset(v_aug[:, :, D:D + 1], 1.0)

                mb_sb = stage.tile([PT, JT], F32, tag="mb_sb")
                nc.sync.dma_start(out=mb_sb, in_=mb_d[h])

                for icp in range(2):          # pairs of i-chunks
                    e_r = er_pool.tile([PT, JT, 2 * ICS], BF16, tag="e_r")
                    for g in range(JT // 2):
                        tT2 = work.tile([PT, 4 * ICS], F32, tag="tT2")
                        for half in range(2):
                            jt = 2 * g + half
                            pt = ps_mm.tile([PT, 2 * ICS], F32, tag="simT")
                            for ici in range(2):
                                ic = 2 * icp + ici
                                nc.tensor.matmul(
                                    pt[:, ici * ICS:(ici + 1) * ICS],
                                    kT_r[:, jt * PT:(jt + 1) * PT],
                                    qT_r[:, ic * ICS:(ic + 1) * ICS],
                                    start=True, stop=True,
                                )
                            nc.scalar.activation(
                                tT2[:, half * 2 * ICS:(half + 1) * 2 * ICS], pt, Tanh,
                                bias=mb_sb[:, jt:jt + 1], scale=1.0 / 400.0,
                            )
                        nc.scalar.activation(
                            e_r[:, 2 * g:2 * g + 2, :].rearrange("p a b -> p (a b)"),
                            tT2, Exp, scale=50.0,
                        )

                    outT_pair = []
                    for _i in range(2):
                        outT_p = ps_acc.tile([D + 1, ICS], F32, tag="outT")
                        outT_pair.append(outT_p)
                    for jt in range(JT):
                        for ici in range(2):
                            esl = slice(ici * ICS, (ici + 1) * ICS)
                            nc.tensor.matmul(
                                outT_pair[ici], v_aug[:, jt, :], e_r[:, jt, esl],
                                start=(jt == 0), stop=(jt == JT - 1),
                            )
                    for ici in range(2):
                        ic = 2 * icp + ici
                        isl = slice(ic * ICS, (ic + 1) * ICS)
                        esl = slice(ici * ICS, (ici + 1) * ICS)
                        outT_ps = outT_pair[ici]
                        outT_sb = outs.tile([D + 1, ICS], F32, tag="outT_sb")
                        nc.vector.tensor_copy(outT_sb, outT_ps)

                        out_sb = outs.tile([PT, IBC, D], F32, tag="out_sb")
                        rc_list = []
                        for ib in range(IBC):
                            outtr = ps_tr.tile([PT, D + 1], F32, tag="attn_ps")
                            nc.tensor.transpose(
                                outtr,
                                outT_sb[:, ib * PT:(ib + 1) * PT],
                                ident65,
                            )
                            rc = recips.tile([PT, 1], F32, tag="recip")
                            nc.vector.reciprocal(rc, outtr[:, D:D + 1])
                            nc.vector.tensor_scalar_mul(
                                out=out_sb[:, ib, :], in0=outtr[:, 0:D], scalar1=rc
                            )
                            rc_list.append(rc)
                        nc.sync.dma_start(
                            out=out_d[h, isl, :].rearrange("(b p) d -> p b d", p=PT),
                            in_=out_sb,
                        )

                        # ---- attn writeout: bf16 transposes + normalize ----
                        for ibp in range(IBC // 2):
                            attn_sb = attn_out.tile([PT, 2, S], F32, tag="attn_sb")
                            for sub in range(2):
                                ib = 2 * ibp + sub
                                for half2 in range(2):
                                    attn_ps = ps_attn.tile([PT, 8 * PT], BF16, tag="attn_ps")
                                    for jj in range(8):
                                        jt = half2 * 8 + jj
                                        nc.tensor.transpose(
                                            attn_ps[:, jj * PT:(jj + 1) * PT],
                                            e_r[:, jt, ici * ICS + ib * PT:
                                                ici * ICS + (ib + 1) * PT],
                                            ident_bf,
                                        )
                                    nc.vector.tensor_scalar_mul(
                                        out=attn_sb[:, sub,
                                                    half2 * 8 * PT:(half2 + 1) * 8 * PT],
                                        in0=attn_ps,
                                        scalar1=rc_list[ib],
                                    )
                            row0 = ic * ICS + 2 * ibp * PT
                            nc.sync.dma_start(
                                out=attn_d[h, row0:row0 + 2 * PT, :].rearrange(
                                    "(b p) j -> p b j", p=PT),
                                in_=attn_sb,
                            )
    nc.compile()
    return nc


def _get_nc():
    global _NC_CACHE
    if _NC_CACHE is None:
        _NC_CACHE = _build()
    return _NC_CACHE


LAST_EXEC_NS = None


def kernel(q, k, v, mask, _trace=False):
    global LAST_EXEC_NS
    q = np.ascontiguousarray(np.asarray(q, dtype=np.float32)).reshape(B * H, S, D)
    k = np.ascontiguousarray(np.asarray(k, dtype=np.float32)).reshape(B * H, S, D)
    v = np.ascontiguousarray(np.asarray(v, dtype=np.float32)).reshape(B * H, S, D)
    mask = np.asarray(mask).astype(bool)

    qT = np.ascontiguousarray(q.transpose(0, 2, 1))                  # [BH, D, S]
    kT = np.ascontiguousarray(k.transpose(0, 2, 1))                  # [BH, D, S]

    # mask bias per head, laid out [128, JT] partition-major per j-tile
    mb = np.where(mask, np.float32(0.0), np.float32(MASK_BIAS))      # [B, S]
    mb = mb.reshape(B, JT, PT).transpose(0, 2, 1)                    # [B, 128, JT]
    mb = np.ascontiguousarray(mb, dtype=np.float32)

    in_maps = []
    for c in range(NCORES):
        heads = range(c * HPC, (c + 1) * HPC)
        in_maps.append({
            "qT": qT[c * HPC:(c + 1) * HPC],
            "kT": kT[c * HPC:(c + 1) * HPC],
            "v": v[c * HPC:(c + 1) * HPC],
            "mb": np.stack([mb[hh // H] for hh in heads]),
        })

    nc = _get_nc()
    res = run_bass_kernel_spmd(
        nc, in_maps, core_ids=list(range(NCORES)), trace=_trace
    )
    LAST_EXEC_NS = res.exec_time_ns

    attn = np.concatenate([r["attn"] for r in res.results]).reshape(B, H, S, S)
    out = np.concatenate([r["out"] for r in res.results]).reshape(B, H, S, D)
    return out, attn
